# revision 15
# baseline (speedup 1.0000x reference)
"""Trainium2 Bass kernel for masked cosine-similarity attention.

reference:
    q_norm = max(||q||, 1e-8); k_norm = max(||k||, 1e-8)
    scores = |q.k / (q_norm k_norm)|           [B,H,K]
    p_attn = exp(where(mask==0, -1e9, scores)) (== mask * exp(scores) in f32)
    out    = p_attn[...,None] * value          [B,H,K,D]
    returns (out, p_attn)

Sharding: batch B=8 -> one batch per NeuronCore; cores fully independent.

Per-core dataflow (H=16 heads, K=2048, D=128), software-pipelined over two
independent K-halves (half = 1024 columns):
  phase A[i]  stream keyT (host-pretransposed [D,K] bf16, 4 heads x half
              per DMA); square on ACT; TensorE contracts d with a merged
              one-hot stationary [D, 48] (cols h: qs_h -> dots rows 0:16;
              cols 32+h: e_h -> ksq rows 32:48), accumulating [48, KH]
              psum over heads. Warm-up matmuls raise the PE HAM clock
              during the initial DMA wait.
  trans[i]    PE-transpose dots/ksq into [128(k%128), 8*H]; compute
              pT = maskT * exp(|dots| * min(exp(-0.5 ln ksq), 1e8))
              (Square/Ln/Exp/Abs/copy share ONE ACT table set);
              PE-transpose back to [H, KH] (p_attn) and strided-transpose
              into P2 [128(k//8), r*H+h] for phase B.
  phase B[i]  stream value as contiguous [128, ...] bf16 (4 heads x half
              per DMA, fully prefetched); o = v * P2 broadcast (DVE TT,
              stride-0 AP); store via SWDGE cast-DMA bf16->f32.
Order: A0, trans0, A1, B0, trans1, B1 — so trans0/B0 overlap A1 on
otherwise-idle engines and the DMA queues never starve.
"""
import numpy as np
import ml_dtypes
from contextlib import ExitStack

B, H, K, D = 8, 16, 2048, 128
KH = K // 2      # 1024 per half
NJH = KH // 128  # 8 k%128-tiles per half
NRH = KH // 128  # 8 r values per k//8 partition in a half
MMN = 512
NCH = KH // MMN  # 2 chunks per half
EPS = 1e-8
WARMUP_MMS = 90

_CACHED = {}


def _build():
    import concourse.tile as tile
    from concourse import bacc, mybir

    f32 = mybir.dt.float32
    bf16 = mybir.dt.bfloat16
    AF = mybir.ActivationFunctionType
    MUL = mybir.AluOpType.mult

    nc = bacc.Bacc("TRN2", target_bir_lowering=False, debug=False)

    keyT_d = nc.dram_tensor("keyT", [H, D, K], bf16, kind="ExternalInput")
    val_d = nc.dram_tensor("value", [H, K, D], bf16, kind="ExternalInput")
    qo1h_d = nc.dram_tensor("qo1h", [D, H, 3 * H], bf16, kind="ExternalInput")
    maskT_d = nc.dram_tensor("maskT", [128, 2, NJH * H], f32, kind="ExternalInput")
    id16_d = nc.dram_tensor("id16", [H, H], f32, kind="ExternalInput")
    id128_d = nc.dram_tensor("id128", [128, 128], f32, kind="ExternalInput")
    out_d = nc.dram_tensor("out", [H, K, D], f32, kind="ExternalOutput")
    pat_d = nc.dram_tensor("p_attn", [H, K], f32, kind="ExternalOutput")

    with tile.TileContext(nc) as tc, ExitStack() as ctx:
        consts = ctx.enter_context(tc.tile_pool(name="consts", bufs=1))
        qo1h = consts.tile([D, H, 3 * H], bf16, tag="qo1h")
        nc.sync.dma_start(qo1h[:], qo1h_d[:])
        maskT = consts.tile([128, 2, NJH * H], f32, tag="maskT")
        nc.scalar.dma_start(maskT[:], maskT_d[:])
        id16 = consts.tile([H, H], f32, tag="id16")
        nc.scalar.dma_start(id16[:], id16_d[:])
        id128 = consts.tile([128, 128], f32, tag="id128")
        nc.scalar.dma_start(id128[:], id128_d[:])

        sm = ctx.enter_context(tc.tile_pool(name="sm", bufs=1))
        keyp = ctx.enter_context(tc.tile_pool(name="keyp", bufs=4))
        sqp = ctx.enter_context(tc.tile_pool(name="sqp", bufs=3))
        vp = ctx.enter_context(tc.tile_pool(name="vp", bufs=7))
        op = ctx.enter_context(tc.tile_pool(name="op", bufs=4))

        def phase_a(half, stats_pool, warmup):
            off = half * KH
            psA = stats_pool.tile([3 * H, KH], f32, tag=f"psA{half}")
            psB = stats_pool.tile([3 * H, KH], f32, tag=f"psB{half}")
            if warmup:
                warm = sm.tile([D, 3 * H], bf16, tag="warm")
                nc.gpsimd.memset(warm[:], 0.0)
                for _ in range(WARMUP_MMS):
                    nc.tensor.matmul(psA[:, 0:3 * H], warm[:], warm[:])
            for h0 in range(0, H, 4):
                kT = keyp.tile([D, 4, KH], bf16, tag="kT")
                nc.sync.dma_start(
                    kT[:],
                    keyT_d[h0:h0 + 4, :, off:off + KH].rearrange("g d k -> d g k"))
                sq = sqp.tile([D, 4, KH], bf16, tag="sq")
                if (h0 // 4) % 2 == 0:
                    nc.scalar.activation(sq[:], kT[:], AF.Square)
                else:
                    nc.vector.tensor_tensor(sq[:], kT[:], kT[:], MUL)
                for g in range(4):
                    h = h0 + g
                    lhs = qo1h[:, h, :]
                    for c in range(NCH):
                        s = slice(c * MMN, (c + 1) * MMN)
                        nc.tensor.matmul(psA[:, s], lhs, kT[:, g, s],
                                         start=(h == 0), stop=(h == H - 1))
                        nc.tensor.matmul(psB[:, s], lhs, sq[:, g, s],
                                         start=(h == 0), stop=(h == H - 1))
            dots_sb = sm.tile([H, KH], f32, tag=f"dots_sb{half}")
            nc.scalar.copy(dots_sb[:], psA[0:H, :])
            ksq_sb = sm.tile([H, KH], f32, tag=f"ksq_sb{half}")
            nc.vector.tensor_copy(ksq_sb[:], psB[2 * H:3 * H, :])
            return dots_sb, ksq_sb

        def trans_smalls(half, post, dots_sb, ksq_sb):
            W = NJH * H  # 128
            dkT_ps = post.tile([128, 2 * W], f32, tag=f"dkT{half}")
            for j in range(NJH):
                nc.tensor.transpose(
                    dkT_ps[:, W + j * H: W + (j + 1) * H],
                    ksq_sb[:, j * 128:(j + 1) * 128], id16[:])
            for j in range(NJH):
                nc.tensor.transpose(
                    dkT_ps[:, j * H:(j + 1) * H],
                    dots_sb[:, j * 128:(j + 1) * 128], id16[:])
            dT = dkT_ps[:, 0:W]
            kq = dkT_ps[:, W:2 * W]

            # rkn = min(ksq^-0.5, 1e8) == 1/max(sqrt(ksq), 1e-8)
            lk = sm.tile([128, W], f32, tag=f"lk{half}")
            nc.scalar.activation(lk[:], kq, AF.Ln)
            ek = sm.tile([128, W], f32, tag=f"ek{half}")
            nc.scalar.activation(ek[:], lk[:], AF.Exp, scale=-0.5)
            absd = sm.tile([128, W], f32, tag=f"absd{half}")
            nc.scalar.activation(absd[:], dT, AF.Abs)
            score = sm.tile([128, W], f32, tag=f"score{half}")
            nc.vector.scalar_tensor_tensor(
                score[:], ek[:], 1e8, absd[:], mybir.AluOpType.min, MUL)
            p0 = sm.tile([128, W], f32, tag=f"p0{half}")
            nc.scalar.activation(p0[:], score[:], AF.Exp)
            pT = sm.tile([128, W], f32, tag=f"pT{half}")
            nc.vector.tensor_mul(pT[:], p0[:], maskT[:, half, :])

            # p_attn rows back to [H, KH]
            pb_ps = post.tile([H, KH], f32, tag=f"pb{half}")
            for j in range(NJH):
                nc.tensor.transpose(
                    pb_ps[:, j * 128:(j + 1) * 128],
                    pT[:, j * H:(j + 1) * H], id128[:])
            p_sb = sm.tile([H, KH], f32, tag=f"p_sb{half}")
            nc.scalar.copy(p_sb[:], pb_ps[:])

            # P2[q, r*H + h] = p[h, 8q + r]
            p2_ps = post.tile([128, NRH * H], f32, tag=f"p2{half}")
            p_r = p_sb[:].rearrange("h (q r) -> h r q", r=NRH)
            for r in range(NRH):
                nc.tensor.transpose(
                    p2_ps[:, r * H:(r + 1) * H], p_r[:, r, :], id16[:])
            P2 = sm.tile([128, NRH * H], f32, tag=f"P2{half}")
            nc.vector.tensor_copy(P2[:], p2_ps[:])
            return P2, p_sb

        def load_values(half):
            off = half * KH
            tiles = []
            for h0 in range(0, H, 4):
                v = vp.tile([128, 4, NRH, D], bf16, tag="v")
                nc.sync.dma_start(
                    v[:],
                    val_d[h0:h0 + 4, off:off + KH, :].rearrange(
                        "g (q r) d -> q g r d", q=128))
                tiles.append(v)
            return tiles

        def mult_store(half, P2, vtiles, gpsimd_tiles=()):
            off = half * KH
            for t, h0 in enumerate(range(0, H, 4)):
                v = vtiles[t]
                o = op.tile([128, 4, NRH, D], bf16, tag="o")
                eng = nc.gpsimd if t in gpsimd_tiles else nc.vector
                for g in range(4):
                    h = h0 + g
                    p_bc = P2[:, h::H][:, :, None].broadcast_to([128, NRH, D])
                    eng.tensor_tensor(o[:, g], v[:, g], p_bc, MUL)
                nc.gpsimd.dma_start(
                    out_d[h0:h0 + 4, off:off + KH, :].rearrange(
                        "g (q r) d -> q g r d", q=128),
                    o[:])

        # ---- pipeline: A0, loadV0, trans0, A1, loadV1, B0, trans1, B1 ----
        with tc.tile_pool(name="statsA", bufs=1, space="PSUM") as statsA:
            d0, k0 = phase_a(0, statsA, warmup=True)
        vt0 = load_values(0)
        with tc.tile_pool(name="postA", bufs=1, space="PSUM") as postA:
            P2_0, psb0 = trans_smalls(0, postA, d0, k0)
            with tc.tile_pool(name="statsB", bufs=1, space="PSUM") as statsB:
                d1, k1 = phase_a(1, statsB, warmup=False)
            vt1 = load_values(1)
            mult_store(0, P2_0, vt0)
        with tc.tile_pool(name="postB", bufs=1, space="PSUM") as postB:
            P2_1, psb1 = trans_smalls(1, postB, d1, k1)
        mult_store(1, P2_1, vt1)

        # p_attn stores (tiny, off critical path)
        nc.scalar.dma_start(pat_d[:, 0:KH], psb0[:])
        nc.scalar.dma_start(pat_d[:, KH:K], psb1[:])

    nc.compile()
    return nc


def _get_nc():
    if "nc" not in _CACHED:
        _CACHED["nc"] = _build()
    return _CACHED["nc"]


def _prep_inputs(query, key, value, mask):
    bf16 = ml_dtypes.bfloat16
    query = np.asarray(query, dtype=np.float32)
    key = np.asarray(key, dtype=np.float32)
    value = np.asarray(value, dtype=np.float32)
    mask = np.asarray(mask)

    q = query[:, :, 0, :]                               # [B,H,D]
    qn = np.maximum(np.sqrt((q * q).sum(-1)), EPS)      # [B,H]
    qs = (q / qn[:, :, None]).astype(bf16)              # [B,H,D]

    # merged one-hot stationary [B, D, H, 48]
    qo1h = np.zeros((B, D, H, 3 * H), dtype=bf16)
    for h in range(H):
        qo1h[:, :, h, h] = qs[:, h, :]
        qo1h[:, :, h, 2 * H + h] = 1.0

    keyT = np.ascontiguousarray(key.transpose(0, 1, 3, 2)).astype(bf16)
    value_bf = value.astype(bf16)

    # maskT[b, p, half, j*H + h] = mask[b, half*KH + j*128 + p]
    m = mask.reshape(B, 2, NJH, 128).transpose(0, 3, 1, 2).astype(np.float32)
    maskT = np.ascontiguousarray(np.repeat(m, H, axis=3))  # [B,128,2,NJH*H]

    id16 = np.eye(H, dtype=np.float32)
    id128 = np.eye(128, dtype=np.float32)

    in_maps = []
    for b in range(B):
        in_maps.append({
            "keyT": keyT[b],
            "value": value_bf[b],
            "qo1h": qo1h[b],
            "maskT": maskT[b],
            "id16": id16,
            "id128": id128,
        })
    return in_maps


def _run(query, key, value, mask, trace=False, tmpdir=None):
    from concourse.bass_utils import run_bass_kernel_spmd

    nc = _get_nc()
    in_maps = _prep_inputs(query, key, value, mask)
    res = run_bass_kernel_spmd(nc, in_maps, core_ids=list(range(B)), trace=trace,
                               tmpdir=tmpdir)
    out = np.stack([res.results[b]["out"] for b in range(B)])
    p_attn = np.stack([res.results[b]["p_attn"] for b in range(B)])
    return (out, p_attn), res


def kernel(query, key, value, mask):
    (out, p_attn), _ = _run(query, key, value, mask, trace=False)
    return out, p_attn


def _ensure_ntff_hook():
    """The container's antenv stub lacks axon_hooks; synthesize it and
    register the ctypes NTFF profile hook against libaxon_pjrt.so."""
    import sys
    import types

    if "antenv.axon_hooks" not in sys.modules:
        mod = types.ModuleType("antenv.axon_hooks")
        holder = [None]
        mod.set_axon_ntff_profile_hook = lambda h: holder.__setitem__(0, h)
        mod.get_axon_ntff_profile_hook = lambda: holder[0]
        sys.modules["antenv.axon_hooks"] = mod
        import antenv

        antenv.axon_hooks = mod
    from antenv.axon_hooks import (
        get_axon_ntff_profile_hook,
        set_axon_ntff_profile_hook,
    )

    if get_axon_ntff_profile_hook() is None:
        from trn_agent_boot.trn_boot import _ntff_profile_via_ctypes

        hook = _ntff_profile_via_ctypes("/opt/axon/libaxon_pjrt.so")
        if hook is not None:
            set_axon_ntff_profile_hook(hook)

    from concourse import bass_utils as bu

    bu.upload_artifacts = lambda tmpdir: f"file://{tmpdir}"


def kernel_profiled(query, key, value, mask, tmpdir=None):
    """Returns ((out, p_attn), exec_time_ns)."""
    try:
        _ensure_ntff_hook()
        (out, p_attn), res = _run(query, key, value, mask, trace=True,
                                  tmpdir=tmpdir)
        return (out, p_attn), res.exec_time_ns
    except Exception as e:
        print(f"[kernel_profiled] trace path failed ({type(e).__name__}: {e}); "
              f"falling back to untraced run")
        (out, p_attn), res = _run(query, key, value, mask, trace=False)
        return (out, p_attn), None


# revision 16
# speedup vs baseline: 1.6358x; 1.6358x over previous
"""Trainium2 Bass kernel for masked cosine-similarity attention.

reference:
    q_norm = max(||q||, 1e-8); k_norm = max(||k||, 1e-8)
    scores = |q.k / (q_norm k_norm)|           [B,H,K]
    p_attn = exp(where(mask==0, -1e9, scores)) (== mask * exp(scores) in f32)
    out    = p_attn[...,None] * value          [B,H,K,D]
    returns (out, p_attn)

Sharding: batch B=8 -> one batch per NeuronCore; cores fully independent.

Per-core dataflow (H=16 heads, K=2048, D=128), software-pipelined over two
independent K-halves (half = 1024 columns):
  phase A[i]  stream keyT (host-pretransposed [D,K] bf16, 4 heads x half
              per DMA); square on ACT; TensorE contracts d with a merged
              one-hot stationary [D, 48] (cols h: qs_h -> dots rows 0:16;
              cols 32+h: e_h -> ksq rows 32:48), accumulating [48, KH]
              psum over heads. Warm-up matmuls raise the PE HAM clock
              during the initial DMA wait.
  trans[i]    PE-transpose dots/ksq into [128(k%128), 8*H]; compute
              pT = maskT * exp(|dots| * min(exp(-0.5 ln ksq), 1e8))
              (Square/Ln/Exp/Abs/copy share ONE ACT table set);
              PE-transpose back to [H, KH] (p_attn) and strided-transpose
              into P2 [128(k//8), r*H+h] for phase B.
  phase B[i]  stream value as contiguous [128, ...] bf16 (4 heads x half
              per DMA, fully prefetched); o = v * P2 broadcast (DVE TT,
              stride-0 AP); store via SWDGE cast-DMA bf16->f32.
Order: A0, trans0, A1, B0, trans1, B1 — so trans0/B0 overlap A1 on
otherwise-idle engines and the DMA queues never starve.
"""
import numpy as np
import ml_dtypes
from contextlib import ExitStack

B, H, K, D = 8, 16, 2048, 128
KH = K // 2      # 1024 per half
NJH = KH // 128  # 8 k%128-tiles per half
NRH = KH // 128  # 8 r values per k//8 partition in a half
MMN = 512
NCH = KH // MMN  # 2 chunks per half
EPS = 1e-8
WARMUP_MMS = 68

_CACHED = {}


def _build():
    import concourse.tile as tile
    from concourse import bacc, mybir

    f32 = mybir.dt.float32
    bf16 = mybir.dt.bfloat16
    AF = mybir.ActivationFunctionType
    MUL = mybir.AluOpType.mult

    nc = bacc.Bacc("TRN2", target_bir_lowering=False, debug=False)

    keyT_d = nc.dram_tensor("keyT", [H, D, K], bf16, kind="ExternalInput")
    val_d = nc.dram_tensor("value", [H, K, D], bf16, kind="ExternalInput")
    qo1h_d = nc.dram_tensor("qo1h", [D, H, 3 * H], bf16, kind="ExternalInput")
    maskT_d = nc.dram_tensor("maskT", [128, 2, NJH * H], f32, kind="ExternalInput")
    id16_d = nc.dram_tensor("id16", [H, H], f32, kind="ExternalInput")
    id128_d = nc.dram_tensor("id128", [128, 128], f32, kind="ExternalInput")
    out_d = nc.dram_tensor("out", [H, K, D], f32, kind="ExternalOutput")
    pat_d = nc.dram_tensor("p_attn", [H, K], f32, kind="ExternalOutput")

    with tile.TileContext(nc) as tc, ExitStack() as ctx:
        consts = ctx.enter_context(tc.tile_pool(name="consts", bufs=1))
        qo1h = consts.tile([D, H, 3 * H], bf16, tag="qo1h")
        nc.sync.dma_start(qo1h[:], qo1h_d[:])
        maskT = consts.tile([128, 2, NJH * H], f32, tag="maskT")
        nc.scalar.dma_start(maskT[:], maskT_d[:])
        id16 = consts.tile([H, H], f32, tag="id16")
        nc.scalar.dma_start(id16[:], id16_d[:])
        id128 = consts.tile([128, 128], f32, tag="id128")
        nc.scalar.dma_start(id128[:], id128_d[:])

        sm = ctx.enter_context(tc.tile_pool(name="sm", bufs=1))
        keyp = ctx.enter_context(tc.tile_pool(name="keyp", bufs=4))
        sqp = ctx.enter_context(tc.tile_pool(name="sqp", bufs=3))
        vp = ctx.enter_context(tc.tile_pool(name="vp", bufs=7))
        op = ctx.enter_context(tc.tile_pool(name="op", bufs=4))

        def phase_a(half, stats_pool, warmup):
            off = half * KH
            psA = stats_pool.tile([3 * H, KH], f32, tag=f"psA{half}")
            psB = stats_pool.tile([3 * H, KH], f32, tag=f"psB{half}")
            if warmup:
                warm = sm.tile([D, 3 * H], bf16, tag="warm")
                nc.gpsimd.memset(warm[:], 0.0)
                for _ in range(WARMUP_MMS):
                    nc.tensor.matmul(psA[:, 0:3 * H], warm[:], warm[:])
            for h0 in range(0, H, 4):
                kT = keyp.tile([D, 4, KH], bf16, tag="kT")
                nc.sync.dma_start(
                    kT[:],
                    keyT_d[h0:h0 + 4, :, off:off + KH].rearrange("g d k -> d g k"))
                sq = sqp.tile([D, 4, KH], bf16, tag="sq")
                if (h0 // 4) % 2 == 0:
                    nc.scalar.activation(sq[:], kT[:], AF.Square)
                else:
                    nc.vector.tensor_tensor(sq[:], kT[:], kT[:], MUL)
                for g in range(4):
                    h = h0 + g
                    lhs = qo1h[:, h, :]
                    for c in range(NCH):
                        s = slice(c * MMN, (c + 1) * MMN)
                        nc.tensor.matmul(psA[:, s], lhs, kT[:, g, s],
                                         start=(h == 0), stop=(h == H - 1))
                        nc.tensor.matmul(psB[:, s], lhs, sq[:, g, s],
                                         start=(h == 0), stop=(h == H - 1))
            dots_sb = sm.tile([H, KH], f32, tag=f"dots_sb{half}")
            nc.scalar.copy(dots_sb[:], psA[0:H, :])
            ksq_sb = sm.tile([H, KH], f32, tag=f"ksq_sb{half}")
            nc.vector.tensor_copy(ksq_sb[:], psB[2 * H:3 * H, :])
            return dots_sb, ksq_sb

        def trans_smalls(half, post, dots_sb, ksq_sb):
            W = NJH * H  # 128
            dkT_ps = post.tile([128, 2 * W], f32, tag=f"dkT{half}")
            for j in range(NJH):
                nc.tensor.transpose(
                    dkT_ps[:, W + j * H: W + (j + 1) * H],
                    ksq_sb[:, j * 128:(j + 1) * 128], id16[:])
            for j in range(NJH):
                nc.tensor.transpose(
                    dkT_ps[:, j * H:(j + 1) * H],
                    dots_sb[:, j * 128:(j + 1) * 128], id16[:])
            dT = dkT_ps[:, 0:W]
            kq = dkT_ps[:, W:2 * W]

            # rkn = min(ksq^-0.5, 1e8) == 1/max(sqrt(ksq), 1e-8)
            lk = sm.tile([128, W], f32, tag=f"lk{half}")
            nc.scalar.activation(lk[:], kq, AF.Ln)
            ek = sm.tile([128, W], f32, tag=f"ek{half}")
            nc.scalar.activation(ek[:], lk[:], AF.Exp, scale=-0.5)
            absd = sm.tile([128, W], f32, tag=f"absd{half}")
            nc.scalar.activation(absd[:], dT, AF.Abs)
            score = sm.tile([128, W], f32, tag=f"score{half}")
            nc.vector.scalar_tensor_tensor(
                score[:], ek[:], 1e8, absd[:], mybir.AluOpType.min, MUL)
            p0 = sm.tile([128, W], f32, tag=f"p0{half}")
            nc.scalar.activation(p0[:], score[:], AF.Exp)
            pT = sm.tile([128, W], f32, tag=f"pT{half}")
            nc.vector.tensor_mul(pT[:], p0[:], maskT[:, half, :])

            # p_attn rows back to [H, KH]
            pb_ps = post.tile([H, KH], f32, tag=f"pb{half}")
            for j in range(NJH):
                nc.tensor.transpose(
                    pb_ps[:, j * 128:(j + 1) * 128],
                    pT[:, j * H:(j + 1) * H], id128[:])
            p_sb = sm.tile([H, KH], f32, tag=f"p_sb{half}")
            nc.scalar.copy(p_sb[:], pb_ps[:])

            # P2[q, r*H + h] = p[h, 8q + r]
            p2_ps = post.tile([128, NRH * H], f32, tag=f"p2{half}")
            p_r = p_sb[:].rearrange("h (q r) -> h r q", r=NRH)
            for r in range(NRH):
                nc.tensor.transpose(
                    p2_ps[:, r * H:(r + 1) * H], p_r[:, r, :], id16[:])
            P2 = sm.tile([128, NRH * H], f32, tag=f"P2{half}")
            nc.vector.tensor_copy(P2[:], p2_ps[:])
            return P2, p_sb

        def load_values(half):
            off = half * KH
            tiles = []
            for h0 in range(0, H, 4):
                v = vp.tile([128, 4, NRH, D], bf16, tag="v")
                nc.sync.dma_start(
                    v[:],
                    val_d[h0:h0 + 4, off:off + KH, :].rearrange(
                        "g (q r) d -> q g r d", q=128))
                tiles.append(v)
            return tiles

        def mult_store(half, P2, vtiles, gpsimd_tiles=()):
            off = half * KH
            for t, h0 in enumerate(range(0, H, 4)):
                v = vtiles[t]
                o = op.tile([128, 4, NRH, D], bf16, tag="o")
                eng = nc.gpsimd if t in gpsimd_tiles else nc.vector
                for g in range(4):
                    h = h0 + g
                    p_bc = P2[:, h::H][:, :, None].broadcast_to([128, NRH, D])
                    eng.tensor_tensor(o[:, g], v[:, g], p_bc, MUL)
                nc.gpsimd.dma_start(
                    out_d[h0:h0 + 4, off:off + KH, :].rearrange(
                        "g (q r) d -> q g r d", q=128),
                    o[:])

        # ---- pipeline: A0, loadV0, trans0, A1, loadV1, B0, trans1, B1 ----
        with tc.tile_pool(name="statsA", bufs=1, space="PSUM") as statsA:
            d0, k0 = phase_a(0, statsA, warmup=True)
        vt0 = load_values(0)
        with tc.tile_pool(name="postA", bufs=1, space="PSUM") as postA:
            P2_0, psb0 = trans_smalls(0, postA, d0, k0)
            with tc.tile_pool(name="statsB", bufs=1, space="PSUM") as statsB:
                d1, k1 = phase_a(1, statsB, warmup=False)
            vt1 = load_values(1)
            mult_store(0, P2_0, vt0)
        with tc.tile_pool(name="postB", bufs=1, space="PSUM") as postB:
            P2_1, psb1 = trans_smalls(1, postB, d1, k1)
        mult_store(1, P2_1, vt1)

        # p_attn stores (tiny, off critical path)
        nc.scalar.dma_start(pat_d[:, 0:KH], psb0[:])
        nc.scalar.dma_start(pat_d[:, KH:K], psb1[:])

    nc.compile()
    return nc


def _get_nc():
    if "nc" not in _CACHED:
        _CACHED["nc"] = _build()
    return _CACHED["nc"]


def _prep_inputs(query, key, value, mask):
    bf16 = ml_dtypes.bfloat16
    query = np.asarray(query, dtype=np.float32)
    key = np.asarray(key, dtype=np.float32)
    value = np.asarray(value, dtype=np.float32)
    mask = np.asarray(mask)

    q = query[:, :, 0, :]                               # [B,H,D]
    qn = np.maximum(np.sqrt((q * q).sum(-1)), EPS)      # [B,H]
    qs = (q / qn[:, :, None]).astype(bf16)              # [B,H,D]

    # merged one-hot stationary [B, D, H, 48]
    qo1h = np.zeros((B, D, H, 3 * H), dtype=bf16)
    for h in range(H):
        qo1h[:, :, h, h] = qs[:, h, :]
        qo1h[:, :, h, 2 * H + h] = 1.0

    keyT = np.ascontiguousarray(key.transpose(0, 1, 3, 2)).astype(bf16)
    value_bf = value.astype(bf16)

    # maskT[b, p, half, j*H + h] = mask[b, half*KH + j*128 + p]
    m = mask.reshape(B, 2, NJH, 128).transpose(0, 3, 1, 2).astype(np.float32)
    maskT = np.ascontiguousarray(np.repeat(m, H, axis=3))  # [B,128,2,NJH*H]

    id16 = np.eye(H, dtype=np.float32)
    id128 = np.eye(128, dtype=np.float32)

    in_maps = []
    for b in range(B):
        in_maps.append({
            "keyT": keyT[b],
            "value": value_bf[b],
            "qo1h": qo1h[b],
            "maskT": maskT[b],
            "id16": id16,
            "id128": id128,
        })
    return in_maps


def _run(query, key, value, mask, trace=False, tmpdir=None):
    from concourse.bass_utils import run_bass_kernel_spmd

    nc = _get_nc()
    in_maps = _prep_inputs(query, key, value, mask)
    res = run_bass_kernel_spmd(nc, in_maps, core_ids=list(range(B)), trace=trace,
                               tmpdir=tmpdir)
    out = np.stack([res.results[b]["out"] for b in range(B)])
    p_attn = np.stack([res.results[b]["p_attn"] for b in range(B)])
    return (out, p_attn), res


def kernel(query, key, value, mask):
    (out, p_attn), _ = _run(query, key, value, mask, trace=False)
    return out, p_attn


def _ensure_ntff_hook():
    """The container's antenv stub lacks axon_hooks; synthesize it and
    register the ctypes NTFF profile hook against libaxon_pjrt.so."""
    import sys
    import types

    if "antenv.axon_hooks" not in sys.modules:
        mod = types.ModuleType("antenv.axon_hooks")
        holder = [None]
        mod.set_axon_ntff_profile_hook = lambda h: holder.__setitem__(0, h)
        mod.get_axon_ntff_profile_hook = lambda: holder[0]
        sys.modules["antenv.axon_hooks"] = mod
        import antenv

        antenv.axon_hooks = mod
    from antenv.axon_hooks import (
        get_axon_ntff_profile_hook,
        set_axon_ntff_profile_hook,
    )

    if get_axon_ntff_profile_hook() is None:
        from trn_agent_boot.trn_boot import _ntff_profile_via_ctypes

        hook = _ntff_profile_via_ctypes("/opt/axon/libaxon_pjrt.so")
        if hook is not None:
            set_axon_ntff_profile_hook(hook)

    from concourse import bass_utils as bu

    bu.upload_artifacts = lambda tmpdir: f"file://{tmpdir}"


def kernel_profiled(query, key, value, mask, tmpdir=None):
    """Returns ((out, p_attn), exec_time_ns)."""
    try:
        _ensure_ntff_hook()
        (out, p_attn), res = _run(query, key, value, mask, trace=True,
                                  tmpdir=tmpdir)
        return (out, p_attn), res.exec_time_ns
    except Exception as e:
        print(f"[kernel_profiled] trace path failed ({type(e).__name__}: {e}); "
              f"falling back to untraced run")
        (out, p_attn), res = _run(query, key, value, mask, trace=False)
        return (out, p_attn), None
